# revision 12
# baseline (speedup 1.0000x reference)
"""Channel-attention (transposed attention) Trainium2 Bass kernel.

Reference computation (per batch b of 8, one NeuronCore each):
    xt   = x[b].reshape(C, N).T                    # [N, C], N = 64*64 = 4096
    qkv  = xt @ w_qkv                              # [N, 3C]
    q, k, v : per-head [N, hd], nh=8, hd=64
    logits_h = k_h.T @ v_h                         # [hd, hd]
    attn_h   = softmax(scale * logits_h, axis=-1)  # scale = hd**-0.5 = 1/8
    out_h    = q_h @ attn_h.T                      # [N, hd]
    y[b] = (concat_h(out_h) @ w_proj + b_proj).T   # [C, N]

Sharding: data-parallel over batch, 1 batch item per core, no collectives.

Algebraic fusion: everything downstream of the softmax is LINEAR in x, so
the q projection, the attention apply, and the output projection collapse
into one [C, C] matrix applied directly to x:

    out^T_h = attn_h @ W_qh^T @ x      (W_qh = w_qkv[:, q cols of head h])
    y^T     = w_proj^T @ out^T + b
            = G @ x + b,   G = w_proj^T @ stack_h(attn_h @ W_qh^T)

This removes the q^T matmuls (128 FD=512 MMs) and the attention-apply
matmuls (32 FD=512 MMs) of the direct formulation, replacing them with
16 PE transposes of w_q (for W_q^T), 4 M-build MMs and 16 G-build MMs.
x stays resident in SBUF (8 MB) and is re-read for the final G @ x pass,
so HBM traffic is unchanged (x once in, y once out, weights once).

Logit pair-packing: heads (2p, 2p+1) share one [128,128] FD=128 matmul
per token tile — lhsT = k-pair columns, rhs = v-pair columns. The two
64x64 diagonal blocks of the output are the real logits; the off-diag
blocks are k_i^T v_j junk that lands in unused PSUM columns. 4 MMs per
token tile instead of 8 col-tiled FD=64 MMs.

Startup: 30 junk warm-up matmuls on a memset tile keep the PE busy from
t~0 so the HAM clock-gate releases (1.2 -> 2.4 GHz) during the DMA
lead-in instead of throttling the first ~3.4us of real matmuls. The DMA
queue is ordered x0/k-sec/v-sec interleaved per chunk so the first kv
accumulation group is runnable after ~0.75 MB of traffic.

The big (free-dim 512) matmuls use float32r (fp32 bytes, FP22 multiply)
— 4x faster than true fp32 at free-dim >= 256, ~6e-5 relative element
precision. The logit matmuls (free-dim 128) stay exact fp32.
"""

import numpy as np

import concourse.bass as bass
import concourse.mybir as mybir
import concourse.tile as tile
from concourse import bass_utils

F32 = mybir.dt.float32
F32R = mybir.dt.float32r
AF = mybir.ActivationFunctionType

# Problem shape (hardcoded per contest contract).
B = 8
C = 512
H = W = 64
N = H * W            # 4096 tokens per batch
NH = 8               # heads
HD = C // NH         # 64
SCALE = HD ** -0.5   # 1/8
KC = C // 128        # 4 contraction chunks of 128 channels
NS = 8               # n-slices of 512 tokens
SL = N // NS         # 512
TT = SL // 128       # 4 token tiles of 128 per slice
HP = NH // 2         # 4 head pairs
WARMUP_MM = 52       # junk matmuls to release the HAM clock gate


def _r(ap):
    return ap.bitcast(F32R)


def _split_multi_waits(nc, max_waits=1):
    """The walrus build in this container encodes at most one sync-wait
    command per instruction (setupSyncWait raises "Too many sync wait
    commands" otherwise — the Tile kernel-tail drain carries several).
    Hoist excess waits onto same-engine NOPs immediately preceding the
    instruction; engine-FIFO order preserves the semantics."""
    n_split = 0
    for bb in nc.main_func.blocks:
        new_insts = []
        for ins in bb.instructions:
            si = ins.sync_info
            waits = list(si.on_wait) if si and si.on_wait else []
            if len(waits) > max_waits:
                extra, keep = waits[:-max_waits], waits[-max_waits:]
                while extra:
                    chunk, extra = extra[:max_waits], extra[max_waits:]
                    nop = mybir.InstNoOp(
                        name=nc.get_next_instruction_name(),
                        ins=[], outs=[],
                        engine=ins.engine,
                        sync_info=mybir.SyncInfo(on_wait=chunk, on_update=[]),
                    )
                    nc.register_instruction(nop)
                    new_insts.append(nop)
                    n_split += 1
                si.on_wait = keep
            new_insts.append(ins)
        bb.instructions[:] = new_insts
    return n_split


def build_nc(reps=1, phases='full'):
    nc = bass.Bass("TRN2", debug=False, num_devices=B)

    x_t = nc.dram_tensor("x", [C, N], F32, kind="ExternalInput")
    wq_t = nc.dram_tensor("w_qkv", [C, 3 * C], F32, kind="ExternalInput")
    wp_t = nc.dram_tensor("w_proj", [C, C], F32, kind="ExternalInput")
    bp_t = nc.dram_tensor("b_proj", [C, 1], F32, kind="ExternalInput")
    y_t = nc.dram_tensor("y", [C, N], F32, kind="ExternalOutput")
    id_t = nc.inline_tensor(np.eye(128, dtype=np.float32), name="id128")

    xd, wqd, wpd, bpd, yd = x_t.ap(), wq_t.ap(), wp_t.ap(), bp_t.ap(), y_t.ap()

    with tile.TileContext(nc) as tc:
        with (
            tc.tile_pool(name="const", bufs=1) as cpool,
            tc.tile_pool(name="xres", bufs=1) as xpool,
            tc.tile_pool(name="soft", bufs=1) as spool,
        ):
            id_sb = cpool.tile([128, 128], F32, tag="id")
            nc.sync.dma_start(id_sb[:], id_t.ap()[:, :])

            # PE warm-up source tile (junk matmuls write into the lg
            # PSUM bank; the first real logit matmul re-zeros it)
            junk_sb = cpool.tile([128, 128], F32, tag="junk")
            nc.gpsimd.memset(junk_sb[:], 0.0)

            # ---- slice-0 x tiles + k-sec + v-sec of w_qkv interleaved
            # per chunk: the first kv accumulation group is runnable
            # after ~0.75 MB of traffic ----
            x_sb = [[xpool.tile([128, SL], F32R, name=f"x{s}_{k}",
                                tag=f"x{s}_{k}") for k in range(KC)]
                    for s in range(NS)]
            wq_sb = [cpool.tile([128, 3 * C], F32R, name=f"wq{k}", tag=f"wq{k}")
                     for k in range(KC)]
            for k in range(KC):
                r = slice(k * 128, (k + 1) * 128)
                nc.sync.dma_start(x_sb[0][k][:], _r(xd[r, 0:SL]))
                nc.sync.dma_start(wq_sb[k][:, 512:1024], _r(wqd[r, 512:1024]))
                nc.sync.dma_start(wq_sb[k][:, 1024:1536], _r(wqd[r, 1024:1536]))

            wp_sb = [cpool.tile([128, C], F32R, name=f"wp{k}", tag=f"wp{k}")
                     for k in range(KC)]
            bp_sb = [cpool.tile([128, 1], F32, name=f"bp{k}", tag=f"bp{k}")
                     for k in range(KC)]
            wqqT_sb = [cpool.tile([128, C], F32R, name=f"wqqT{j}", tag=f"wqqT{j}")
                       for j in range(KC)]
            gT_sb = [cpool.tile([128, C], F32R, name=f"gT{c}", tag=f"gT{c}")
                     for c in range(KC)]
            h_sb = [cpool.tile([128, C], F32R, name=f"h{p}", tag=f"h{p}")
                    for p in range(HP)]

            for _rep in range(reps):
                _build_one_pass(nc, tc, spool, wq_sb, wp_sb, bp_sb, id_sb,
                                wqqT_sb, gT_sb, h_sb, junk_sb, x_sb, xd, yd,
                                wqd, wpd, bpd, first_rep=(_rep == 0),
                                phases=phases)
    _split_multi_waits(nc)
    return nc


def _build_one_pass(nc, tc, spool, wq_sb, wp_sb, bp_sb, id_sb, wqqT_sb,
                    gT_sb, h_sb, junk_sb, x_sb, xd, yd, wqd, wpd, bpd,
                    first_rep=True, phases="full"):
    # phases: prefix gating for attribution benchmarks
    lvl = ["dma", "qkv", "logits", "soft", "attn", "full"].index(phases)

    # logits accumulator: one PSUM bank, 4 pair-blocks of [128,128].
    # Block p cols [128p, 128p+128): rows/cols (0:64, 0:64) = head 2p
    # logits, (64:128, 64:128) = head 2p+1; off-diag blocks are junk.
    with tc.tile_pool(name="lgp", bufs=1, space="PSUM") as lgpool:
        lg = lgpool.tile([128, HP * 128], F32, tag="lg")
        if first_rep:
            # PE warm-up: junk matmuls into the lg bank from t~0 release
            # the HAM clock gate during the DMA lead-in. The junk-cell
            # memsets below then zero what the first logit group needs.
            for _ in range(WARMUP_MM):
                nc.tensor.matmul(lg[:, 0:128], junk_sb[:], junk_sb[:],
                                 start=True, stop=True)
        # The never-written off-diagonal cells of each pair block must
        # read as 0.0 at softmax time (reduce_max runs over the full
        # block): zero them once, before the logit accumulation group.
        for p in range(HP):
            for par in range(2):
                nc.vector.memset(
                    lg[par * 64:(par + 1) * 64,
                       p * 128 + (1 - par) * 64:p * 128 + (2 - par) * 64],
                    0.0,
                )

        # ================= Phase A: KV + logit accumulation =========
        with (
            tc.tile_pool(name="kvs", bufs=3) as kvpool,
            tc.tile_pool(name="kvp", bufs=2, space="PSUM") as kvpsum,
            tc.tile_pool(name="wtp", bufs=2, space="PSUM") as wtpsum,
        ):
            for ns in range(NS):
                if first_rep and ns >= 1:
                    # stream the rest of x behind the startup-critical
                    # loads; q-sec / w_proj / b_proj slot in where they
                    # are not yet needed
                    for k in range(KC):
                        nc.sync.dma_start(
                            x_sb[ns][k][:],
                            _r(xd[k * 128:(k + 1) * 128,
                                  ns * SL:(ns + 1) * SL]),
                        )
                    if ns == 2:
                        for k in range(KC):
                            r = slice(k * 128, (k + 1) * 128)
                            nc.sync.dma_start(wq_sb[k][:, 0:512],
                                              _r(wqd[r, 0:512]))
                    if ns == 4:
                        for k in range(KC):
                            r = slice(k * 128, (k + 1) * 128)
                            nc.sync.dma_start(wp_sb[k][:], _r(wpd[r, :]))
                            nc.sync.dma_start(bp_sb[k][:], bpd[r, :])
                xs = x_sb[ns]
                if lvl < 1:
                    continue
                # --- k,v token tiles + logit accumulation ---
                for t in range(TT):
                    kvp = kvpsum.tile([128, 2 * C], F32, tag="kv_ps")
                    for k in range(KC):
                        xk = xs[k][:, t * 128:(t + 1) * 128]
                        nc.tensor.matmul(
                            kvp[:, 0:512], xk, wq_sb[k][:, 512:1024],
                            start=(k == 0), stop=(k == KC - 1),
                        )
                        nc.tensor.matmul(
                            kvp[:, 512:1024], xk, wq_sb[k][:, 1024:1536],
                            start=(k == 0), stop=(k == KC - 1),
                        )
                    kv_sb = kvpool.tile([128, 2 * C], F32, tag="kv_sb")
                    nc.vector.tensor_copy(kv_sb[:], kvp[:])
                    if lvl < 2:
                        continue
                    first = ns == 0 and t == 0
                    last = ns == NS - 1 and t == TT - 1
                    for h in range(NH):
                        p, par = divmod(h, 2)
                        # col-packed FD=64: par=0/par=1 target different
                        # PE column groups and run concurrently. Writes
                        # accumulate into the diag sub-blocks only; the
                        # memset zeros in the off-diag cells survive
                        # (accumulation start does not clear stored
                        # bytes of never-written cells).
                        nc.tensor.matmul(
                            lg[par * 64:(par + 1) * 64,
                               p * 128 + par * 64:p * 128 + par * 64 + 64],
                            kv_sb[:, h * 64:(h + 1) * 64],
                            kv_sb[:, 512 + h * 64:512 + (h + 1) * 64],
                            start=first and h == 0,
                            stop=last and h == NH - 1,
                        )

                if ns == 3 and first_rep:
                    # W_q^T via PE transposes — q-sec has landed by now;
                    # runs in the PE stream between kv groups.
                    # wqqT[j][:, k*128:(k+1)*128] = wq[k chunk, j blk]^T
                    for j in range(KC):
                        for k in range(KC):
                            wt = wtpsum.tile([128, 128], F32, tag="wt")
                            nc.tensor.transpose(
                                wt[:],
                                wq_sb[k][:, j * 128:(j + 1) * 128].bitcast(F32),
                                id_sb[:],
                            )
                            nc.scalar.activation(
                                wqqT_sb[j][:, k * 128:(k + 1) * 128],
                                wt[:], AF.Copy,
                            )

        if lvl < 3:
            return
        # ================= Phase B: softmax ==========================
        # Per pair p: one reduce_max over the FULL [128,128] block (the
        # junk cells read the 0.0 written before accumulation; a shift
        # of max(real_max, 0) is still an exact softmax shift), one exp
        # over the full block, then zero the off-diag exp(junk) cells,
        # one reduce_sum per block, and fold diag(1/s) into bd rows.
        bda = spool.tile([128, HP * 128], F32, tag="bda")
        mx = spool.tile([128, HP], F32, tag="mx")
        bias = spool.tile([128, HP], F32, tag="bias")
        ssum = spool.tile([128, HP], F32, tag="ssum")
        recip = spool.tile([128, HP], F32, tag="recip")
        bd2 = [spool.tile([128, 128], F32R, name=f"bd2_{p}", tag=f"bd2_{p}")
               for p in range(HP)]

        for p in range(HP):
            blk = slice(p * 128, (p + 1) * 128)
            nc.vector.reduce_max(
                mx[:, p:p + 1], lg[:, blk],
                axis=mybir.AxisListType.X,
            )
            nc.vector.tensor_scalar_mul(
                bias[:, p:p + 1], mx[:, p:p + 1], -SCALE)
            nc.scalar.activation(
                bda[:, blk], lg[:, blk], AF.Exp,
                bias=bias[:, p:p + 1], scale=SCALE,
            )

    # lg pool closed; rest of softmax runs on SBUF tiles
    for p in range(HP):
        blk = slice(p * 128, (p + 1) * 128)
        # zero the exp(junk) off-diag cells before the row sums
        for par in range(2):
            nc.gpsimd.memset(
                bda[par * 64:(par + 1) * 64,
                    p * 128 + (1 - par) * 64:p * 128 + (2 - par) * 64],
                0.0,
            )
        nc.vector.reduce_sum(
            ssum[:, p:p + 1], bda[:, blk],
            axis=mybir.AxisListType.X,
        )
        nc.vector.reciprocal(recip[:, p:p + 1], ssum[:, p:p + 1])
        # bd2 = diag(1/s) . blockdiag(exp) for pair p
        nc.vector.tensor_scalar_mul(bd2[p][:], bda[:, blk],
                                    recip[:, p:p + 1])

    # ---- H = E^T diag(r) wp (contracts d; bd2 is lhsT directly, no
    # transposes), then G^T = W_q H (contracts e via wqqT) ----
    with (
        tc.tile_pool(name="hps", bufs=2, space="PSUM") as hpsum,
        tc.tile_pool(name="gps", bufs=1, space="PSUM") as gpsum,
    ):
        # interleaved per pair: H_p, then its 4 G^T contributions, so
        # the PE consumes each pair's softmax chain as soon as it lands
        gp = [gpsum.tile([128, C], F32, name=f"g_ps{c}", tag=f"g_ps{c}")
              for c in range(KC)]
        for p in range(HP):
            hp_ = hpsum.tile([128, C], F32, tag="h_ps")
            nc.tensor.matmul(hp_[:], bd2[p][:], wp_sb[p][:],
                             start=True, stop=True)
            nc.scalar.activation(h_sb[p][:], hp_[:], AF.Copy)
            for cc in range(KC):
                nc.tensor.matmul(
                    gp[cc][:], wqqT_sb[p][:, cc * 128:(cc + 1) * 128],
                    h_sb[p][:],
                    start=(p == 0), stop=(p == HP - 1),
                )
        for cc in range(KC):
            nc.scalar.activation(gT_sb[cc][:], gp[cc][:], AF.Copy)

    if lvl < 4:
        return
    # ================= Phase D: y^T = G @ x + b =====================
    # slice-pairs share one [128,1024] psum tile (2 banks, 2 groups):
    # one ACT (bias-add) and one 4KB-per-partition DMA per (pair, co)
    with (
        tc.tile_pool(name="ys", bufs=3) as ypool,
        tc.tile_pool(name="yp", bufs=3, space="PSUM") as ypsum,
    ):
        for nsp in range(NS // 2):
            ns0, ns1 = 2 * nsp, 2 * nsp + 1
            for co in range(KC):
                yp = ypsum.tile([128, 2 * SL], F32, tag="y_ps")
                for k in range(KC):
                    nc.tensor.matmul(
                        yp[:, 0:SL],
                        gT_sb[k][:, co * 128:(co + 1) * 128],
                        x_sb[ns0][k][:],
                        start=(k == 0),
                        stop=(k == KC - 1),
                    )
                    nc.tensor.matmul(
                        yp[:, SL:2 * SL],
                        gT_sb[k][:, co * 128:(co + 1) * 128],
                        x_sb[ns1][k][:],
                        start=(k == 0),
                        stop=(k == KC - 1),
                    )
                ysb = ypool.tile([128, 2 * SL], F32, tag="y_sb")
                nc.scalar.activation(
                    ysb[:], yp[:], AF.Identity,
                    bias=bp_sb[co][:, 0:1], scale=1.0,
                )
                nc.sync.dma_start(
                    yd[co * 128:(co + 1) * 128,
                       nsp * 2 * SL:(nsp + 1) * 2 * SL],
                    ysb[:],
                )


_NC_CACHE = None


def kernel(x, w_qkv, w_proj, b_proj, num_heads):
    x = np.ascontiguousarray(np.asarray(x, dtype=np.float32))
    w_qkv = np.ascontiguousarray(np.asarray(w_qkv, dtype=np.float32))
    w_proj = np.ascontiguousarray(np.asarray(w_proj, dtype=np.float32))
    b_proj = np.ascontiguousarray(np.asarray(b_proj, dtype=np.float32))
    assert int(num_heads) == NH
    assert x.shape == (B, C, H, W)

    xs = x.reshape(B, C, N)
    bp2 = b_proj.reshape(C, 1)
    in_maps = [
        {"x": xs[b], "w_qkv": w_qkv, "w_proj": w_proj, "b_proj": bp2}
        for b in range(B)
    ]
    global _NC_CACHE
    if _NC_CACHE is None:
        _NC_CACHE = build_nc()
    res = bass_utils.run_bass_kernel_spmd(_NC_CACHE, in_maps, list(range(B)))
    y = np.stack([res.results[b]["y"] for b in range(B)])
    return y.reshape(B, C, H, W).astype(np.float32)


if __name__ == "__main__":
    nc = build_nc()
    n_inst = sum(len(bb.instructions) for bb in nc.main_func.blocks)
    print(f"built OK, {n_inst} instructions")


# revision 13
# speedup vs baseline: 1.2138x; 1.2138x over previous
"""Channel-attention (transposed attention) Trainium2 Bass kernel.

Reference computation (per batch b of 8, one NeuronCore each):
    xt   = x[b].reshape(C, N).T                    # [N, C], N = 64*64 = 4096
    qkv  = xt @ w_qkv                              # [N, 3C]
    q, k, v : per-head [N, hd], nh=8, hd=64
    logits_h = k_h.T @ v_h                         # [hd, hd]
    attn_h   = softmax(scale * logits_h, axis=-1)  # scale = hd**-0.5 = 1/8
    out_h    = q_h @ attn_h.T                      # [N, hd]
    y[b] = (concat_h(out_h) @ w_proj + b_proj).T   # [C, N]

Sharding: data-parallel over batch, 1 batch item per core, no collectives.

Algebraic fusion: everything downstream of the softmax is LINEAR in x, so
the q projection, the attention apply, and the output projection collapse
into one [C, C] matrix applied directly to x:

    out^T_h = attn_h @ W_qh^T @ x      (W_qh = w_qkv[:, q cols of head h])
    y^T     = w_proj^T @ out^T + b
            = G @ x + b,   G = w_proj^T @ stack_h(attn_h @ W_qh^T)

This removes the q^T matmuls (128 FD=512 MMs) and the attention-apply
matmuls (32 FD=512 MMs) of the direct formulation, replacing them with
16 PE transposes of w_q (for W_q^T), 4 M-build MMs and 16 G-build MMs.
x stays resident in SBUF (8 MB) and is re-read for the final G @ x pass,
so HBM traffic is unchanged (x once in, y once out, weights once).

Logit pair-packing: heads (2p, 2p+1) share one [128,128] FD=128 matmul
per token tile — lhsT = k-pair columns, rhs = v-pair columns. The two
64x64 diagonal blocks of the output are the real logits; the off-diag
blocks are k_i^T v_j junk that lands in unused PSUM columns. 4 MMs per
token tile instead of 8 col-tiled FD=64 MMs.

Startup: 30 junk warm-up matmuls on a memset tile keep the PE busy from
t~0 so the HAM clock-gate releases (1.2 -> 2.4 GHz) during the DMA
lead-in instead of throttling the first ~3.4us of real matmuls. The DMA
queue is ordered x0/k-sec/v-sec interleaved per chunk so the first kv
accumulation group is runnable after ~0.75 MB of traffic.

The big (free-dim 512) matmuls use float32r (fp32 bytes, FP22 multiply)
— 4x faster than true fp32 at free-dim >= 256, ~6e-5 relative element
precision. The logit matmuls (free-dim 128) stay exact fp32.
"""

import numpy as np

import concourse.bass as bass
import concourse.mybir as mybir
import concourse.tile as tile
from concourse import bass_utils

F32 = mybir.dt.float32
F32R = mybir.dt.float32r
AF = mybir.ActivationFunctionType

# Problem shape (hardcoded per contest contract).
B = 8
C = 512
H = W = 64
N = H * W            # 4096 tokens per batch
NH = 8               # heads
HD = C // NH         # 64
SCALE = HD ** -0.5   # 1/8
KC = C // 128        # 4 contraction chunks of 128 channels
NS = 8               # n-slices of 512 tokens
SL = N // NS         # 512
TT = SL // 128       # 4 token tiles of 128 per slice
HP = NH // 2         # 4 head pairs
WARMUP_MM = 30       # junk matmuls to release the HAM clock gate


def _r(ap):
    return ap.bitcast(F32R)


def _split_multi_waits(nc, max_waits=1):
    """The walrus build in this container encodes at most one sync-wait
    command per instruction (setupSyncWait raises "Too many sync wait
    commands" otherwise — the Tile kernel-tail drain carries several).
    Hoist excess waits onto same-engine NOPs immediately preceding the
    instruction; engine-FIFO order preserves the semantics."""
    n_split = 0
    for bb in nc.main_func.blocks:
        new_insts = []
        for ins in bb.instructions:
            si = ins.sync_info
            waits = list(si.on_wait) if si and si.on_wait else []
            if len(waits) > max_waits:
                extra, keep = waits[:-max_waits], waits[-max_waits:]
                while extra:
                    chunk, extra = extra[:max_waits], extra[max_waits:]
                    nop = mybir.InstNoOp(
                        name=nc.get_next_instruction_name(),
                        ins=[], outs=[],
                        engine=ins.engine,
                        sync_info=mybir.SyncInfo(on_wait=chunk, on_update=[]),
                    )
                    nc.register_instruction(nop)
                    new_insts.append(nop)
                    n_split += 1
                si.on_wait = keep
            new_insts.append(ins)
        bb.instructions[:] = new_insts
    return n_split


def build_nc(reps=1, phases='full'):
    nc = bass.Bass("TRN2", debug=False, num_devices=B)

    x_t = nc.dram_tensor("x", [C, N], F32, kind="ExternalInput")
    wq_t = nc.dram_tensor("w_qkv", [C, 3 * C], F32, kind="ExternalInput")
    wp_t = nc.dram_tensor("w_proj", [C, C], F32, kind="ExternalInput")
    bp_t = nc.dram_tensor("b_proj", [C, 1], F32, kind="ExternalInput")
    y_t = nc.dram_tensor("y", [C, N], F32, kind="ExternalOutput")
    id_t = nc.inline_tensor(np.eye(128, dtype=np.float32), name="id128")

    xd, wqd, wpd, bpd, yd = x_t.ap(), wq_t.ap(), wp_t.ap(), bp_t.ap(), y_t.ap()

    with tile.TileContext(nc) as tc:
        with (
            tc.tile_pool(name="const", bufs=1) as cpool,
            tc.tile_pool(name="xres", bufs=1) as xpool,
            tc.tile_pool(name="soft", bufs=1) as spool,
        ):
            id_sb = cpool.tile([128, 128], F32, tag="id")
            nc.sync.dma_start(id_sb[:], id_t.ap()[:, :])

            # PE warm-up source tile (junk matmuls write into the lg
            # PSUM bank; the first real logit matmul re-zeros it)
            junk_sb = cpool.tile([128, 128], F32, tag="junk")
            nc.gpsimd.memset(junk_sb[:], 0.0)

            # ---- slice-0 x tiles + k-sec + v-sec of w_qkv interleaved
            # per chunk: the first kv accumulation group is runnable
            # after ~0.75 MB of traffic ----
            x_sb = [[xpool.tile([128, SL], F32R, name=f"x{s}_{k}",
                                tag=f"x{s}_{k}") for k in range(KC)]
                    for s in range(NS)]
            wq_sb = [cpool.tile([128, 3 * C], F32R, name=f"wq{k}", tag=f"wq{k}")
                     for k in range(KC)]
            for k in range(KC):
                r = slice(k * 128, (k + 1) * 128)
                nc.sync.dma_start(x_sb[0][k][:], _r(xd[r, 0:SL]))
                nc.sync.dma_start(wq_sb[k][:, 512:1024], _r(wqd[r, 512:1024]))
                nc.sync.dma_start(wq_sb[k][:, 1024:1536], _r(wqd[r, 1024:1536]))

            wp_sb = [cpool.tile([128, C], F32R, name=f"wp{k}", tag=f"wp{k}")
                     for k in range(KC)]
            bp_sb = [cpool.tile([128, 1], F32, name=f"bp{k}", tag=f"bp{k}")
                     for k in range(KC)]
            wqqT_sb = [cpool.tile([128, C], F32R, name=f"wqqT{j}", tag=f"wqqT{j}")
                       for j in range(KC)]
            gT_sb = [cpool.tile([128, C], F32R, name=f"gT{c}", tag=f"gT{c}")
                     for c in range(KC)]
            h_sb = [cpool.tile([128, C], F32R, name=f"h{p}", tag=f"h{p}")
                    for p in range(HP)]

            for _rep in range(reps):
                _build_one_pass(nc, tc, spool, wq_sb, wp_sb, bp_sb, id_sb,
                                wqqT_sb, gT_sb, h_sb, junk_sb, x_sb, xd, yd,
                                wqd, wpd, bpd, first_rep=(_rep == 0),
                                phases=phases)
    _split_multi_waits(nc)
    return nc


def _build_one_pass(nc, tc, spool, wq_sb, wp_sb, bp_sb, id_sb, wqqT_sb,
                    gT_sb, h_sb, junk_sb, x_sb, xd, yd, wqd, wpd, bpd,
                    first_rep=True, phases="full"):
    # phases: prefix gating for attribution benchmarks
    lvl = ["dma", "qkv", "logits", "soft", "attn", "full"].index(phases)

    # logits accumulator: one PSUM bank, 4 pair-blocks of [128,128].
    # Block p cols [128p, 128p+128): rows/cols (0:64, 0:64) = head 2p
    # logits, (64:128, 64:128) = head 2p+1; off-diag blocks are junk.
    with tc.tile_pool(name="lgp", bufs=1, space="PSUM") as lgpool:
        lg = lgpool.tile([128, HP * 128], F32, tag="lg")
        if first_rep:
            # PE warm-up: junk matmuls into the lg bank from t~0 release
            # the HAM clock gate during the DMA lead-in. The junk-cell
            # memsets below then zero what the first logit group needs.
            for _ in range(WARMUP_MM):
                nc.tensor.matmul(lg[:, 0:128], junk_sb[:], junk_sb[:],
                                 start=True, stop=True)
        # The never-written off-diagonal cells of each pair block must
        # read as 0.0 at softmax time (reduce_max runs over the full
        # block): zero them once, before the logit accumulation group.
        for p in range(HP):
            for par in range(2):
                nc.vector.memset(
                    lg[par * 64:(par + 1) * 64,
                       p * 128 + (1 - par) * 64:p * 128 + (2 - par) * 64],
                    0.0,
                )

        # ================= Phase A: KV + logit accumulation =========
        with (
            tc.tile_pool(name="kvs", bufs=3) as kvpool,
            tc.tile_pool(name="kvp", bufs=2, space="PSUM") as kvpsum,
            tc.tile_pool(name="wtp", bufs=2, space="PSUM") as wtpsum,
        ):
            for ns in range(NS):
                if first_rep and ns >= 1:
                    # stream the rest of x behind the startup-critical
                    # loads; q-sec / w_proj / b_proj slot in where they
                    # are not yet needed
                    for k in range(KC):
                        nc.sync.dma_start(
                            x_sb[ns][k][:],
                            _r(xd[k * 128:(k + 1) * 128,
                                  ns * SL:(ns + 1) * SL]),
                        )
                    if ns == 2:
                        for k in range(KC):
                            r = slice(k * 128, (k + 1) * 128)
                            nc.sync.dma_start(wq_sb[k][:, 0:512],
                                              _r(wqd[r, 0:512]))
                    if ns == 4:
                        for k in range(KC):
                            r = slice(k * 128, (k + 1) * 128)
                            nc.sync.dma_start(wp_sb[k][:], _r(wpd[r, :]))
                            nc.sync.dma_start(bp_sb[k][:], bpd[r, :])
                xs = x_sb[ns]
                if lvl < 1:
                    continue
                # --- k,v token tiles + logit accumulation ---
                for t in range(TT):
                    kvp = kvpsum.tile([128, 2 * C], F32, tag="kv_ps")
                    for k in range(KC):
                        xk = xs[k][:, t * 128:(t + 1) * 128]
                        nc.tensor.matmul(
                            kvp[:, 0:512], xk, wq_sb[k][:, 512:1024],
                            start=(k == 0), stop=(k == KC - 1),
                        )
                        nc.tensor.matmul(
                            kvp[:, 512:1024], xk, wq_sb[k][:, 1024:1536],
                            start=(k == 0), stop=(k == KC - 1),
                        )
                    kv_sb = kvpool.tile([128, 2 * C], F32, tag="kv_sb")
                    nc.vector.tensor_copy(kv_sb[:], kvp[:])
                    if lvl < 2:
                        continue
                    first = ns == 0 and t == 0
                    last = ns == NS - 1 and t == TT - 1
                    for h in range(NH):
                        p, par = divmod(h, 2)
                        # col-packed FD=64: par=0/par=1 target different
                        # PE column groups and run concurrently. Writes
                        # accumulate into the diag sub-blocks only; the
                        # memset zeros in the off-diag cells survive
                        # (accumulation start does not clear stored
                        # bytes of never-written cells).
                        nc.tensor.matmul(
                            lg[par * 64:(par + 1) * 64,
                               p * 128 + par * 64:p * 128 + par * 64 + 64],
                            kv_sb[:, h * 64:(h + 1) * 64],
                            kv_sb[:, 512 + h * 64:512 + (h + 1) * 64],
                            start=first and h == 0,
                            stop=last and h == NH - 1,
                        )

                if ns == 3 and first_rep:
                    # W_q^T via PE transposes — q-sec has landed by now;
                    # runs in the PE stream between kv groups.
                    # wqqT[j][:, k*128:(k+1)*128] = wq[k chunk, j blk]^T
                    for j in range(KC):
                        for k in range(KC):
                            wt = wtpsum.tile([128, 128], F32, tag="wt")
                            nc.tensor.transpose(
                                wt[:],
                                wq_sb[k][:, j * 128:(j + 1) * 128].bitcast(F32),
                                id_sb[:],
                            )
                            nc.scalar.activation(
                                wqqT_sb[j][:, k * 128:(k + 1) * 128],
                                wt[:], AF.Copy,
                            )

        if lvl < 3:
            return
        # ================= Phase B: softmax ==========================
        # Per pair p: one reduce_max over the FULL [128,128] block (the
        # junk cells read the 0.0 written before accumulation; a shift
        # of max(real_max, 0) is still an exact softmax shift), one exp
        # over the full block, then zero the off-diag exp(junk) cells,
        # one reduce_sum per block, and fold diag(1/s) into bd rows.
        bda = spool.tile([128, HP * 128], F32, tag="bda")
        mx = spool.tile([128, HP], F32, tag="mx")
        bias = spool.tile([128, HP], F32, tag="bias")
        ssum = spool.tile([128, HP], F32, tag="ssum")
        recip = spool.tile([128, HP], F32, tag="recip")
        bd2 = [spool.tile([128, 128], F32R, name=f"bd2_{p}", tag=f"bd2_{p}")
               for p in range(HP)]

        for p in range(HP):
            blk = slice(p * 128, (p + 1) * 128)
            nc.vector.reduce_max(
                mx[:, p:p + 1], lg[:, blk],
                axis=mybir.AxisListType.X,
            )
            nc.vector.tensor_scalar_mul(
                bias[:, p:p + 1], mx[:, p:p + 1], -SCALE)
            nc.scalar.activation(
                bda[:, blk], lg[:, blk], AF.Exp,
                bias=bias[:, p:p + 1], scale=SCALE,
            )

    # lg pool closed; rest of softmax runs on SBUF tiles
    for p in range(HP):
        blk = slice(p * 128, (p + 1) * 128)
        # zero the exp(junk) off-diag cells before the row sums
        for par in range(2):
            nc.gpsimd.memset(
                bda[par * 64:(par + 1) * 64,
                    p * 128 + (1 - par) * 64:p * 128 + (2 - par) * 64],
                0.0,
            )
        nc.vector.reduce_sum(
            ssum[:, p:p + 1], bda[:, blk],
            axis=mybir.AxisListType.X,
        )
        nc.vector.reciprocal(recip[:, p:p + 1], ssum[:, p:p + 1])
        # bd2 = diag(1/s) . blockdiag(exp) for pair p
        nc.vector.tensor_scalar_mul(bd2[p][:], bda[:, blk],
                                    recip[:, p:p + 1])

    # ---- H = E^T diag(r) wp (contracts d; bd2 is lhsT directly, no
    # transposes), then G^T = W_q H (contracts e via wqqT) ----
    with (
        tc.tile_pool(name="hps", bufs=2, space="PSUM") as hpsum,
        tc.tile_pool(name="gps", bufs=1, space="PSUM") as gpsum,
    ):
        # interleaved per pair: H_p, then its 4 G^T contributions, so
        # the PE consumes each pair's softmax chain as soon as it lands
        gp = [gpsum.tile([128, C], F32, name=f"g_ps{c}", tag=f"g_ps{c}")
              for c in range(KC)]
        for p in range(HP):
            hp_ = hpsum.tile([128, C], F32, tag="h_ps")
            nc.tensor.matmul(hp_[:], bd2[p][:], wp_sb[p][:],
                             start=True, stop=True)
            nc.scalar.activation(h_sb[p][:], hp_[:], AF.Copy)
            for cc in range(KC):
                nc.tensor.matmul(
                    gp[cc][:], wqqT_sb[p][:, cc * 128:(cc + 1) * 128],
                    h_sb[p][:],
                    start=(p == 0), stop=(p == HP - 1),
                )
        for cc in range(KC):
            nc.scalar.activation(gT_sb[cc][:], gp[cc][:], AF.Copy)

    if lvl < 4:
        return
    # ================= Phase D: y^T = G @ x + b =====================
    # slice-pairs share one [128,1024] psum tile (2 banks, 2 groups):
    # one ACT (bias-add) and one 4KB-per-partition DMA per (pair, co)
    with (
        tc.tile_pool(name="ys", bufs=3) as ypool,
        tc.tile_pool(name="yp", bufs=3, space="PSUM") as ypsum,
    ):
        for nsp in range(NS // 2):
            ns0, ns1 = 2 * nsp, 2 * nsp + 1
            for co in range(KC):
                yp = ypsum.tile([128, 2 * SL], F32, tag="y_ps")
                for k in range(KC):
                    nc.tensor.matmul(
                        yp[:, 0:SL],
                        gT_sb[k][:, co * 128:(co + 1) * 128],
                        x_sb[ns0][k][:],
                        start=(k == 0),
                        stop=(k == KC - 1),
                    )
                    nc.tensor.matmul(
                        yp[:, SL:2 * SL],
                        gT_sb[k][:, co * 128:(co + 1) * 128],
                        x_sb[ns1][k][:],
                        start=(k == 0),
                        stop=(k == KC - 1),
                    )
                ysb = ypool.tile([128, 2 * SL], F32, tag="y_sb")
                nc.scalar.activation(
                    ysb[:], yp[:], AF.Identity,
                    bias=bp_sb[co][:, 0:1], scale=1.0,
                )
                nc.sync.dma_start(
                    yd[co * 128:(co + 1) * 128,
                       nsp * 2 * SL:(nsp + 1) * 2 * SL],
                    ysb[:],
                )


_NC_CACHE = None


def kernel(x, w_qkv, w_proj, b_proj, num_heads):
    x = np.ascontiguousarray(np.asarray(x, dtype=np.float32))
    w_qkv = np.ascontiguousarray(np.asarray(w_qkv, dtype=np.float32))
    w_proj = np.ascontiguousarray(np.asarray(w_proj, dtype=np.float32))
    b_proj = np.ascontiguousarray(np.asarray(b_proj, dtype=np.float32))
    assert int(num_heads) == NH
    assert x.shape == (B, C, H, W)

    xs = x.reshape(B, C, N)
    bp2 = b_proj.reshape(C, 1)
    in_maps = [
        {"x": xs[b], "w_qkv": w_qkv, "w_proj": w_proj, "b_proj": bp2}
        for b in range(B)
    ]
    global _NC_CACHE
    if _NC_CACHE is None:
        _NC_CACHE = build_nc()
    res = bass_utils.run_bass_kernel_spmd(_NC_CACHE, in_maps, list(range(B)))
    y = np.stack([res.results[b]["y"] for b in range(B)])
    return y.reshape(B, C, H, W).astype(np.float32)


if __name__ == "__main__":
    nc = build_nc()
    n_inst = sum(len(bb.instructions) for bb in nc.main_func.blocks)
    print(f"built OK, {n_inst} instructions")
